# revision 5
# baseline (speedup 1.0000x reference)
"""CondConv (per-sample dynamic conv) Trainium2 Bass kernel.

Reference computation (per sample b):
    gap     = mean(x[b], spatial)                    # [C]
    r       = sigmoid(fc_w @ gap + fc_b)             # [E]
    comb    = sum_e r[e] * kernel_weights[e]         # [O, I, 3, 3]
    y[b]    = conv2d(x[b], comb, pad=1)              # [O, H, W]

Sharding: data-parallel over batch, 4 samples per core on 8 cores.
Expert kernels + fc params replicated to every core.

This version computes the conv with Winograd F(2,3) along H (the kh
dimension): 1.5x fewer PE MACs than direct conv. Per output row-pair
(tile t, rows 2t/2t+1), with d = x[2t-1..2t+2]:
    in  taus: t0=d0-d2  t1=d1+d2  t2=d2-d1  t3=d1-d3          (gpsimd)
    wt  taus: g0=w[0]  g1=.5(w0+w1+w2)  g2=.5(w0-w1+w2)  g3=w[2]
    m[tau]   = sum_{ci,kw} g[tau,kw] * in[tau, col+kw-1]      (PE, PSUM)
    y[2t]    = m0+m1+m2      y[2t+1] = m1-m2-m3               (DVE)
No SBUF padding anywhere: row edges are handled by special-casing the
two taus that touch them (t=0: t0=-d2; t=27: t3=d1), col edges by
accumulating the kw=0/2 taps into shifted partial PSUM ranges.

Per-core dataflow:
  - x[s] arrives via gpsimd cast-DMA (fp32 HBM -> bf16 SBUF, contiguous)
  - GAP via DVE tensor_reduce on the bf16 image; routing chain is
    PE (fc matmul) -> ACT sigmoid -> PE (eye broadcast) -> ACT copy
  - synthesis of the combined 3x3 kernel on DVE (tensor_scalar 4x +
    tensor_tensor 2x over the bf16 expert stack), then the tau weight
    transform (5 small DVE ops) produces the g1/g2 blocks
  - conv: per (blk of 7 tiles, oh) 4 PSUM tiles [128,392] each
    accumulate 6 bf16 matmuls (2 ci x 3 kw, edge kw shifted/partial)
  - output transform: 4 DVE tensor_tensor ops PSUM->SBUF, then HWDGE
    DMA of 14 contiguous rows to HBM
  - W load split across both HWDGE rings (sync+scalar), oh=0 experts
    first, so synthesis never waits on the tail of the 9.4MB transfer

Software pipeline: input transforms (gpsimd), synthesis + weight taus
(DVE) and routing for sample s+1 are interleaved into the 8 conv
blocks of sample s, so PE never starves in steady state.
"""

import numpy as np
import ml_dtypes

B, C, H, W = 32, 256, 56, 56
E = 8
N_CORES = 8
BL = B // N_CORES          # local batch per core
HWU = H * W                # 3136
NT = H // 2                # 28 winograd row tiles
BLKT = 7                   # row tiles per conv block
NBLK = NT // BLKT          # 4 blocks
NF = BLKT * W              # 392 matmul free dim
OIN = 128                  # output channels per half
EBLK = 2 * 2 * 9 * OIN     # per-partition free elems per expert = 4608
OHBLK = EBLK // 2          # per (oh) block = 2304

_CACHE = {}


def _build():
    import concourse.bacc as bacc
    import concourse.mybir as mybir
    import concourse.tile as tile
    from contextlib import ExitStack

    dt = mybir.dt
    AF = mybir.ActivationFunctionType
    Alu = mybir.AluOpType
    Ax = mybir.AxisListType

    nc = bacc.Bacc(
        "TRN2",
        target_bir_lowering=False,
        debug=False,
        enable_asserts=False,
        num_devices=N_CORES,
    )
    x_d = nc.dram_tensor("x", [BL, C, H, W], dt.float32, kind="ExternalInput")
    w_d = nc.dram_tensor("wp", [128, E * EBLK], dt.bfloat16, kind="ExternalInput")
    fcw_d = nc.dram_tensor("fcw", [C, E], dt.float32, kind="ExternalInput")
    fcb_d = nc.dram_tensor("fcb", [E, 1], dt.float32, kind="ExternalInput")
    eye_d = nc.dram_tensor("eye", [E, E], dt.float32, kind="ExternalInput")
    y_d = nc.dram_tensor("y", [BL, C, H, W], dt.float32, kind="ExternalOutput")

    with tile.TileContext(nc) as tc:
        with ExitStack() as ctx:
            cpool = ctx.enter_context(tc.tile_pool(name="consts", bufs=1))
            xvpool = ctx.enter_context(tc.tile_pool(name="xvs", bufs=2))
            xtpool = ctx.enter_context(tc.tile_pool(name="xts", bufs=5))
            cbkpool = ctx.enter_context(tc.tile_pool(name="cbks", bufs=2))
            wtpool = ctx.enter_context(tc.tile_pool(name="wts", bufs=2))
            opool = ctx.enter_context(tc.tile_pool(name="outs", bufs=3))
            spool = ctx.enter_context(tc.tile_pool(name="small", bufs=2))
            pspool = ctx.enter_context(tc.tile_pool(name="cpsum", bufs=6, space="PSUM"))
            psmall = ctx.enter_context(tc.tile_pool(name="spsum", bufs=1, space="PSUM"))

            w_sb = cpool.tile([128, E * EBLK], dt.bfloat16)
            fcw_sb = cpool.tile([128, 2 * E], dt.float32)
            fcb_sb = cpool.tile([E, 1], dt.float32)
            eye_sb = cpool.tile([E, E], dt.float32)

            xvs, xts, gaps, rbs, cbks, wt12s = {}, {}, {}, {}, {}, {}

            def load_consts():
                for ci in range(2):
                    nc.sync.dma_start(
                        out=fcw_sb[:, ci * E : (ci + 1) * E],
                        in_=fcw_d.ap()[ci * 128 : (ci + 1) * 128, :],
                    )
                nc.scalar.dma_start(out=fcb_sb[:], in_=fcb_d.ap())
                nc.scalar.dma_start(out=eye_sb[:], in_=eye_d.ap())

            def load_w():
                # oh=0 blocks for all experts first; alternate HWDGE rings
                # so the 9.4MB spreads over both queues.
                for oh in range(2):
                    for e in range(E):
                        lo = e * EBLK + oh * OHBLK
                        eng = nc.sync if e % 2 == 0 else nc.scalar
                        eng.dma_start(
                            out=w_sb[:, lo : lo + OHBLK],
                            in_=w_d.ap()[:, lo : lo + OHBLK],
                        )

            def stage(s):
                # gpsimd SWDGE cast-DMA: fp32 HBM -> bf16 SBUF, contiguous
                xv = xvpool.tile([128, 2 * HWU], dt.bfloat16, tag="xv", name=f"xv{s}")
                xvs[s] = xv
                for ci in range(2):
                    nc.gpsimd.dma_start(
                        out=xv[:, ci * HWU : (ci + 1) * HWU],
                        in_=x_d.ap()[s, ci * 128 : (ci + 1) * 128, :, :],
                    )

            def gap_route(s):
                xv = xvs[s]
                g = spool.tile([128, 2], dt.float32, tag="gap")
                gaps[s] = g
                for ci in range(2):
                    nc.vector.tensor_reduce(
                        out=g[:, ci : ci + 1],
                        in_=xv[:, ci * HWU : (ci + 1) * HWU],
                        axis=Ax.X,
                        op=Alu.add,
                    )
                pl = psmall.tile([E, 1], dt.float32, tag="pl")
                for ci in range(2):
                    nc.tensor.matmul(
                        pl[:],
                        lhsT=fcw_sb[:, ci * E : (ci + 1) * E],
                        rhs=g[:, ci : ci + 1],
                        start=(ci == 0),
                        stop=(ci == 1),
                    )
                rr = spool.tile([E, 1], dt.float32, tag="rr")
                nc.scalar.activation(
                    out=rr[:], in_=pl[:], func=AF.Sigmoid, bias=fcb_sb[:], scale=1.0
                )
                prb = psmall.tile([128, E], dt.float32, tag="prb")
                nc.tensor.matmul(
                    prb[:],
                    lhsT=rr[:].broadcast_to([E, 128]),
                    rhs=eye_sb[:],
                    start=True,
                    stop=True,
                )
                rb = spool.tile([128, E], dt.float32, tag="rb")
                nc.scalar.activation(out=rb[:], in_=prb[:], func=AF.Copy)
                rbs[s] = rb

            def synth(s, oh, elo, ehi):
                if s not in cbks:
                    cbks[s] = cbkpool.tile([128, EBLK], dt.bfloat16, tag="cbk", name=f"cbk{s}")
                cb = cbks[s]
                rb = rbs[s]
                dstc = cb[:, oh * OHBLK : (oh + 1) * OHBLK]
                for e in range(elo, ehi):
                    src = w_sb[:, e * EBLK + oh * OHBLK : e * EBLK + (oh + 1) * OHBLK]
                    if e == 0:
                        nc.vector.tensor_scalar_mul(dstc, src, rb[:, 0:1])
                    else:
                        tmp = spool.tile([128, OHBLK], dt.bfloat16, tag="stmp")
                        nc.vector.tensor_scalar_mul(tmp[:], src, rb[:, e : e + 1])
                        nc.vector.tensor_tensor(
                            out=dstc, in0=tmp[:], in1=dstc, op=Alu.add
                        )

            def wtau(s, oh):
                # g1 = .5*(w0+w1+w2), g2 = .5*(w0-w1+w2) over the kh axis
                if s not in wt12s:
                    wt12s[s] = wtpool.tile([128, 2 * 2 * 2 * 3 * OIN], dt.bfloat16, tag="wt12", name=f"wt12_{s}")
                cbv = cbks[s].rearrange(
                    "p (oh ci kh kw oin) -> p oh ci kh kw oin",
                    oh=2, ci=2, kh=3, kw=3, oin=OIN,
                )
                wtv = wt12s[s].rearrange(
                    "p (oh ci tu kw oin) -> p oh ci tu kw oin",
                    oh=2, ci=2, tu=2, kw=3, oin=OIN,
                )
                tmp = spool.tile([128, 2 * 3 * OIN], dt.bfloat16, tag="wtmp")
                hlf = spool.tile([128, 2 * 3 * OIN], dt.bfloat16, tag="whlf")
                tv = tmp[:].rearrange("p (ci kw oin) -> p ci kw oin", ci=2, kw=3, oin=OIN)
                hv = hlf[:].rearrange("p (ci kw oin) -> p ci kw oin", ci=2, kw=3, oin=OIN)
                nc.vector.tensor_tensor(
                    out=tv, in0=cbv[:, oh, :, 0], in1=cbv[:, oh, :, 2], op=Alu.add
                )
                nc.vector.tensor_scalar_mul(hv, cbv[:, oh, :, 1], 0.5)
                nc.vector.tensor_scalar_mul(tv, tv, 0.5)
                nc.vector.tensor_tensor(
                    out=wtv[:, oh, :, 0], in0=tv, in1=hv, op=Alu.add
                )
                nc.vector.tensor_tensor(
                    out=wtv[:, oh, :, 1], in0=tv, in1=hv, op=Alu.subtract
                )

            def transform(s, blk):
                # input taus for row tiles t = 7*blk .. 7*blk+6, on gpsimd
                xt = xtpool.tile([128, 2 * 4 * BLKT * W], dt.bfloat16, tag="xt", name=f"xt{s}_{blk}")
                xts[(s, blk)] = xt
                xtv = xt.rearrange(
                    "p (ci tu t w) -> p ci tu t w", ci=2, tu=4, t=BLKT, w=W
                )
                for ci in range(2):
                    xr = xvs[s][:, ci * HWU : (ci + 1) * HWU].rearrange(
                        "p (t two w) -> p t two w", t=NT, two=2, w=W
                    )
                    t0 = BLKT * blk
                    t1 = t0 + BLKT
                    # tau0 = d0 - d2 = x[2t-1] - x[2t+1]
                    if blk == 0:
                        nc.gpsimd.tensor_scalar_mul(
                            xtv[:, ci, 0, 0:1], xr[:, 0:1, 1], -1.0
                        )
                        nc.gpsimd.tensor_tensor(
                            out=xtv[:, ci, 0, 1:BLKT],
                            in0=xr[:, 0 : BLKT - 1, 1],
                            in1=xr[:, 1:BLKT, 1],
                            op=Alu.subtract,
                        )
                    else:
                        nc.gpsimd.tensor_tensor(
                            out=xtv[:, ci, 0],
                            in0=xr[:, t0 - 1 : t1 - 1, 1],
                            in1=xr[:, t0:t1, 1],
                            op=Alu.subtract,
                        )
                    # tau1 = d1 + d2 ; tau2 = d2 - d1
                    nc.gpsimd.tensor_tensor(
                        out=xtv[:, ci, 1],
                        in0=xr[:, t0:t1, 0],
                        in1=xr[:, t0:t1, 1],
                        op=Alu.add,
                    )
                    nc.gpsimd.tensor_tensor(
                        out=xtv[:, ci, 2],
                        in0=xr[:, t0:t1, 1],
                        in1=xr[:, t0:t1, 0],
                        op=Alu.subtract,
                    )
                    # tau3 = d1 - d3 = x[2t] - x[2t+2]
                    if blk == NBLK - 1:
                        nc.gpsimd.tensor_tensor(
                            out=xtv[:, ci, 3, 0 : BLKT - 1],
                            in0=xr[:, t0 : t1 - 1, 0],
                            in1=xr[:, t0 + 1 : t1, 0],
                            op=Alu.subtract,
                        )
                        nc.gpsimd.tensor_scalar_mul(
                            xtv[:, ci, 3, BLKT - 1 : BLKT], xr[:, t1 - 1 : t1, 0], 1.0
                        )
                    else:
                        nc.gpsimd.tensor_tensor(
                            out=xtv[:, ci, 3],
                            in0=xr[:, t0:t1, 0],
                            in1=xr[:, t0 + 1 : t1 + 1, 0],
                            op=Alu.subtract,
                        )

            def conv_blk(s, blk, oh):
                cbv = cbks[s].rearrange(
                    "p (oh ci kh kw oin) -> p oh ci kh kw oin",
                    oh=2, ci=2, kh=3, kw=3, oin=OIN,
                )
                wtv = wt12s[s].rearrange(
                    "p (oh ci tu kw oin) -> p oh ci tu kw oin",
                    oh=2, ci=2, tu=2, kw=3, oin=OIN,
                )
                xtv = xts[(s, blk)].rearrange(
                    "p (ci tu t w) -> p ci tu t w", ci=2, tu=4, t=BLKT, w=W
                )

                def lhsT(tau, ci, kw):
                    if tau == 0:
                        return cbv[:, oh, ci, 0, kw]
                    if tau == 3:
                        return cbv[:, oh, ci, 2, kw]
                    return wtv[:, oh, ci, tau - 1, kw]

                pss = []
                for tau in range(4):
                    ps = pspool.tile([128, NF], dt.float32, tag="ps")
                    pss.append(ps)
                    psv = ps.rearrange("p (t w) -> p t w", t=BLKT, w=W)
                    # kw=1 full-range first (start) and last (stop); the
                    # edge taps accumulate into shifted partial ranges.
                    nc.tensor.matmul(
                        ps[:], lhsT=lhsT(tau, 0, 1), rhs=xtv[:, 0, tau],
                        start=True, stop=False,
                    )
                    for ci in range(2):
                        nc.tensor.matmul(
                            psv[:, :, 1:W],
                            lhsT=lhsT(tau, ci, 0),
                            rhs=xtv[:, ci, tau, :, 0 : W - 1],
                            start=False, stop=False,
                            skip_group_check=True,
                        )
                        nc.tensor.matmul(
                            psv[:, :, 0 : W - 1],
                            lhsT=lhsT(tau, ci, 2),
                            rhs=xtv[:, ci, tau, :, 1:W],
                            start=False, stop=False,
                            skip_group_check=True,
                        )
                    nc.tensor.matmul(
                        ps[:], lhsT=lhsT(tau, 1, 1), rhs=xtv[:, 1, tau],
                        start=False, stop=True,
                    )

                # output transform on DVE: y[2t]=m0+m1+m2, y[2t+1]=m1-m2-m3
                ys = opool.tile([128, 2 * BLKT * W], dt.float32, tag="ys")
                ysv = ys.rearrange("p (t two w) -> p t two w", t=BLKT, two=2, w=W)
                p0 = pss[0].rearrange("p (t w) -> p t w", t=BLKT, w=W)
                p1 = pss[1].rearrange("p (t w) -> p t w", t=BLKT, w=W)
                p2 = pss[2].rearrange("p (t w) -> p t w", t=BLKT, w=W)
                p3 = pss[3].rearrange("p (t w) -> p t w", t=BLKT, w=W)
                # DVE can read only ONE input from PSUM per op: seed both
                # parities with an ACT copy of m1, then accumulate in place.
                nc.scalar.activation(out=ysv[:, :, 0], in_=p1[:], func=AF.Copy)
                nc.vector.tensor_tensor(
                    out=ysv[:, :, 0], in0=ysv[:, :, 0], in1=p0[:], op=Alu.add
                )
                nc.vector.tensor_tensor(
                    out=ysv[:, :, 0], in0=ysv[:, :, 0], in1=p2[:], op=Alu.add
                )
                nc.scalar.activation(out=ysv[:, :, 1], in_=p1[:], func=AF.Copy)
                nc.vector.tensor_tensor(
                    out=ysv[:, :, 1], in0=ysv[:, :, 1], in1=p2[:], op=Alu.subtract
                )
                nc.vector.tensor_tensor(
                    out=ysv[:, :, 1], in0=ysv[:, :, 1], in1=p3[:], op=Alu.subtract
                )
                r0 = 2 * BLKT * blk
                eng = nc.sync if (blk + oh) % 2 == 0 else nc.scalar
                eng.dma_start(
                    out=y_d.ap()[s, oh * 128 : (oh + 1) * 128, r0 : r0 + 2 * BLKT, :],
                    in_=ys[:].rearrange("p (r w) -> p r w", r=2 * BLKT, w=W),
                )

            def conv_sample(s):
                nxt = s + 1 if s + 1 < BL else None
                for blk in range(NBLK):
                    for oh in range(2):
                        conv_blk(s, blk, oh)
                        if nxt is None:
                            continue
                        slot = 2 * blk + oh
                        if slot == 0:
                            if nxt + 1 < BL:
                                stage(nxt + 1)
                            transform(nxt, 0)
                        elif slot == 1:
                            transform(nxt, 1)
                        elif slot == 2:
                            transform(nxt, 2)
                        elif slot == 3:
                            gap_route(nxt)
                        elif slot == 4:
                            transform(nxt, 3)
                            synth(nxt, 0, 0, 5)
                        elif slot == 5:
                            synth(nxt, 0, 5, E)
                            wtau(nxt, 0)
                        elif slot == 6:
                            synth(nxt, 1, 0, 6)
                        elif slot == 7:
                            synth(nxt, 1, 6, E)
                            wtau(nxt, 1)

            # ---- software-pipelined emission ----
            load_consts()
            stage(0)
            load_w()
            gap_route(0)
            stage(1)
            synth(0, 0, 0, E)
            wtau(0, 0)
            transform(0, 0)
            transform(0, 1)
            synth(0, 1, 0, E)
            wtau(0, 1)
            transform(0, 2)
            transform(0, 3)
            gap_route(1)
            for s in range(BL):
                conv_sample(s)

    nc.compile()
    return nc


def _get_nc():
    if "nc" not in _CACHE:
        _CACHE["nc"] = _build()
    return _CACHE["nc"]


def _pack_inputs(x, kernel_weights, fc_w, fc_b):
    # w layout per partition p (= i % 128): [e, oh, ci, kh, kw, oin], bf16
    a = np.asarray(kernel_weights, np.float32).reshape(E, 2, 128, 2, 128, 3, 3)
    # dims: e, oh, oin, ci, p, kh, kw -> p, e, oh, ci, kh, kw, oin
    a = np.ascontiguousarray(a.transpose(4, 0, 1, 3, 5, 6, 2)).reshape(128, E * EBLK)
    wp = a.astype(ml_dtypes.bfloat16)
    fcw_t = np.ascontiguousarray(np.asarray(fc_w, np.float32).T / float(H * W))
    fcb2 = np.ascontiguousarray(np.asarray(fc_b, np.float32).reshape(E, 1))
    eye = np.eye(E, dtype=np.float32)
    x = np.ascontiguousarray(np.asarray(x, np.float32))
    in_maps = []
    for i in range(N_CORES):
        in_maps.append(
            {
                "x": x[i * BL : (i + 1) * BL],
                "wp": wp,
                "fcw": fcw_t,
                "fcb": fcb2,
                "eye": eye,
            }
        )
    return in_maps


def _run(x, kernel_weights, fc_w, fc_b, trace=False):
    from concourse.bass_utils import run_bass_kernel_spmd

    nc = _get_nc()
    in_maps = _pack_inputs(x, kernel_weights, fc_w, fc_b)
    res = run_bass_kernel_spmd(nc, in_maps, core_ids=list(range(N_CORES)), trace=trace)
    y = np.concatenate([res.results[i]["y"] for i in range(N_CORES)], axis=0)
    return np.ascontiguousarray(y.astype(np.float32)), res


def kernel(x, kernel_weights, fc_w, fc_b):
    y, _ = _run(x, kernel_weights, fc_w, fc_b, trace=False)
    return y


def kernel_traced(x, kernel_weights, fc_w, fc_b):
    y, res = _run(x, kernel_weights, fc_w, fc_b, trace=True)
    return y, res


# revision 9
# speedup vs baseline: 1.2304x; 1.2304x over previous
"""CondConv (per-sample dynamic conv) Trainium2 Bass kernel.

Reference computation (per sample b):
    gap     = mean(x[b], spatial)                    # [C]
    r       = sigmoid(fc_w @ gap + fc_b)             # [E]
    comb    = sum_e r[e] * kernel_weights[e]         # [O, I, 3, 3]
    y[b]    = conv2d(x[b], comb, pad=1)              # [O, H, W]

Sharding: data-parallel over batch, 4 samples per core on 8 cores.
Expert kernels + fc params replicated to every core.

Direct bf16 conv (measured: PE issues one 448-free matmul per ~199ns,
so the 2*7*18 = 252 matmuls/sample are the machine floor; Winograd
trades below-floor PE work for DVE work it cannot afford). The version
differences vs the naive pipeline are all about keeping PE fed from
t=~20us on:

  - x arrives via gpsimd SWDGE cast-DMA (fp32 HBM -> bf16 SBUF,
    contiguous, no padding). Conv edge taps are handled by accumulating
    shifted partial PSUM ranges instead of padding, so there is no
    fp32 staging tile, no ACT cast pass, and no memsets.
  - W (9.4MB) is pushed as 32 (oh, ci, e) chunks alternating across
    BOTH HWDGE rings (sync+scalar), (oh0,ci0) chunks first: the first
    synthesis needs only 2.4MB landed rather than the full 9.4MB.
  - each conv oh-pass opens all 7 PSUM row-groups and runs a ci0 tap
    pass (63 matmuls) then a ci1 pass; sample 0's first synthesis is
    ci-split so conv starts after half a synthesis (~8us) instead of a
    full one.
  - GAP rides a DVE tensor_scalar copy via accum_out (~1us/half);
    routing is PE (fc matmul) -> ACT sigmoid -> PE (eye broadcast).
  - synthesis (DVE tensor_scalar 4x + tensor_tensor 2x over the bf16
    expert stack) is emitted ci-half at a time into 4 filler slots per
    sample, sized so every half lands >=8us before its conv pass.
  - PSUM->SBUF output copies on ACT, output DMA alternates rings.
"""

import numpy as np
import ml_dtypes

B, C, H, W = 32, 256, 56, 56
E = 8
N_CORES = 8
BL = B // N_CORES          # local batch per core
HWU = H * W                # 3136
ROWS = 8                   # output rows per PSUM group
NG = H // ROWS             # 7 groups per oh pass
NF = ROWS * W              # 448 matmul free dim
OIN = 128                  # output channels per half
EBLK = 2 * 2 * 9 * OIN     # per-partition free elems per expert = 4608
OHBLK = EBLK // 2          # per (oh) block = 2304
CIBLK = OHBLK // 2         # per (oh, ci) block = 1152

_CACHE = {}


def _build():
    import concourse.bacc as bacc
    import concourse.mybir as mybir
    import concourse.tile as tile
    from contextlib import ExitStack

    dt = mybir.dt
    AF = mybir.ActivationFunctionType
    Alu = mybir.AluOpType

    nc = bacc.Bacc(
        "TRN2",
        target_bir_lowering=False,
        debug=False,
        enable_asserts=False,
        num_devices=N_CORES,
    )
    x_d = nc.dram_tensor("x", [BL, C, H, W], dt.float32, kind="ExternalInput")
    w_d = nc.dram_tensor("wp", [128, E * EBLK], dt.bfloat16, kind="ExternalInput")
    fcw_d = nc.dram_tensor("fcw", [C, E], dt.float32, kind="ExternalInput")
    fcb_d = nc.dram_tensor("fcb", [E, 1], dt.float32, kind="ExternalInput")
    eye_d = nc.dram_tensor("eye", [E, E], dt.float32, kind="ExternalInput")
    y_d = nc.dram_tensor("y", [BL, C, H, W], dt.float32, kind="ExternalOutput")

    with tile.TileContext(nc) as tc:
        with ExitStack() as ctx:
            cpool = ctx.enter_context(tc.tile_pool(name="consts", bufs=1))
            xvpool = ctx.enter_context(tc.tile_pool(name="xvs", bufs=3))
            cbpool = ctx.enter_context(tc.tile_pool(name="cbs", bufs=2))
            opool = ctx.enter_context(tc.tile_pool(name="outs", bufs=3))
            spool = ctx.enter_context(tc.tile_pool(name="small", bufs=2))
            pspool = ctx.enter_context(tc.tile_pool(name="cpsum", bufs=7, space="PSUM"))
            psmall = ctx.enter_context(tc.tile_pool(name="spsum", bufs=1, space="PSUM"))

            w_sb = cpool.tile([128, E * EBLK], dt.bfloat16)
            fcw_sb = cpool.tile([128, 2 * E], dt.float32)
            fcb_sb = cpool.tile([E, 1], dt.float32)
            eye_sb = cpool.tile([E, E], dt.float32)
            gscr = cpool.tile([128, HWU], dt.bfloat16)

            xvs, gaps, rbs, cbs = {}, {}, {}, {}

            def load_consts():
                for ci in range(2):
                    nc.sync.dma_start(
                        out=fcw_sb[:, ci * E : (ci + 1) * E],
                        in_=fcw_d.ap()[ci * 128 : (ci + 1) * 128, :],
                    )
                nc.scalar.dma_start(out=fcb_sb[:], in_=fcb_d.ap())
                nc.scalar.dma_start(out=eye_sb[:], in_=eye_d.ap())

            def load_w():
                # (oh, ci, e) chunks, oh0-ci0 first, alternating HWDGE rings
                k = 0
                for oh in range(2):
                    for ci in range(2):
                        for e in range(E):
                            lo = e * EBLK + oh * OHBLK + ci * CIBLK
                            eng = nc.sync if k % 2 == 0 else nc.scalar
                            eng.dma_start(
                                out=w_sb[:, lo : lo + CIBLK],
                                in_=w_d.ap()[:, lo : lo + CIBLK],
                            )
                            k += 1

            def stage(s):
                if s >= BL:
                    return
                xv = xvpool.tile([128, 2 * HWU], dt.bfloat16, tag="xv", name=f"xv{s}")
                xvs[s] = xv
                for ci in range(2):
                    nc.gpsimd.dma_start(
                        out=xv[:, ci * HWU : (ci + 1) * HWU],
                        in_=x_d.ap()[s, ci * 128 : (ci + 1) * 128, :, :],
                    )

            def gap_route(s):
                xv = xvs[s]
                g = spool.tile([128, 2], dt.float32, tag="gap")
                gaps[s] = g
                # gap sum rides a DVE tensor_scalar copy via accum_out
                for ci in range(2):
                    nc.vector.tensor_scalar(
                        out=gscr[:],
                        in0=xv[:, ci * HWU : (ci + 1) * HWU],
                        scalar1=1.0,
                        scalar2=0.0,
                        op0=Alu.mult,
                        op1=Alu.add,
                        accum_out=g[:, ci : ci + 1],
                    )
                prt = psmall.tile([128, E], dt.float32, tag="prt", name=f"prt{s}")
                for ci in range(2):
                    nc.tensor.matmul(
                        prt[0:E, 0:1],
                        lhsT=fcw_sb[:, ci * E : (ci + 1) * E],
                        rhs=g[:, ci : ci + 1],
                        start=(ci == 0),
                        stop=(ci == 1),
                    )
                rr = spool.tile([E, 1], dt.float32, tag="rr")
                nc.scalar.activation(
                    out=rr[:], in_=prt[0:E, 0:1], func=AF.Sigmoid, bias=fcb_sb[:],
                    scale=1.0,
                )
                nc.tensor.matmul(
                    prt[:],
                    lhsT=rr[:].broadcast_to([E, 128]),
                    rhs=eye_sb[:],
                    start=True,
                    stop=True,
                )
                rb = spool.tile([128, E], dt.float32, tag="rb")
                nc.scalar.activation(out=rb[:], in_=prt[:], func=AF.Copy)
                rbs[s] = rb

            def synth(s, oh, ci):
                # combined[oh,ci] = sum_e r_e * W_e[oh,ci]  (bf16, DVE)
                if s not in cbs:
                    cbs[s] = cbpool.tile([128, EBLK], dt.bfloat16, tag="cb",
                                         name=f"cb{s}")
                cb = cbs[s]
                rb = rbs[s]
                lo = oh * OHBLK + ci * CIBLK
                dstc = cb[:, lo : lo + CIBLK]
                for e in range(E):
                    src = w_sb[:, e * EBLK + lo : e * EBLK + lo + CIBLK]
                    if e == 0:
                        nc.vector.tensor_scalar_mul(dstc, src, rb[:, 0:1])
                    else:
                        tmp = spool.tile([128, CIBLK], dt.bfloat16, tag="stmp")
                        nc.vector.tensor_scalar_mul(tmp[:], src, rb[:, e : e + 1])
                        nc.vector.tensor_tensor(
                            out=dstc, in0=tmp[:], in1=dstc, op=Alu.add
                        )

            def conv_oh(s, oh, fillers):
                # 7 PSUM groups opened by a ci0 tap pass, closed by ci1.
                # Edge taps (kh at first/last group, kw 0/2 everywhere) write
                # shifted partial ranges; (kh=1,kw=1) is full-range and
                # carries the start/stop flags.
                xv = xvs[s]
                cb = cbs[s]
                xvv = [
                    xv[:, ci * HWU : (ci + 1) * HWU].rearrange(
                        "p (h w) -> p h w", h=H, w=W
                    )
                    for ci in range(2)
                ]
                pss = [
                    pspool.tile([128, NF], dt.float32, tag="ps", name=f"ps{s}_{oh}_{g}")
                    for g in range(NG)
                ]

                def taps(ci, order):
                    for g in range(NG):
                        ps = pss[g]
                        psv = ps.rearrange("p (r w) -> p r w", r=ROWS, w=W)
                        r0 = g * ROWS
                        for kh, kw in order:
                            lo = oh * OHBLK + ci * CIBLK + (kh * 3 + kw) * OIN
                            full = not (
                                kw != 1
                                or (g == 0 and kh == 0)
                                or (g == NG - 1 and kh == 2)
                            )
                            if full:
                                nc.tensor.matmul(
                                    ps[:],
                                    lhsT=cb[:, lo : lo + OIN],
                                    rhs=xvv[ci][:, r0 + kh - 1 : r0 + kh - 1 + ROWS, :],
                                    start=(ci == 0 and kh == 1),
                                    stop=(ci == 1 and kh == 1),
                                )
                                continue
                            rl = 1 if (g == 0 and kh == 0) else 0
                            rh = ROWS - 1 if (g == NG - 1 and kh == 2) else ROWS
                            cl = 1 if kw == 0 else 0
                            ch = W - 1 if kw == 2 else W
                            nc.tensor.matmul(
                                psv[:, rl:rh, cl:ch],
                                lhsT=cb[:, lo : lo + OIN],
                                rhs=xvv[ci][
                                    :,
                                    r0 + rl + kh - 1 : r0 + rh + kh - 1,
                                    cl + kw - 1 : ch + kw - 1,
                                ],
                                start=False,
                                stop=False,
                                skip_group_check=True,
                            )

                # ci0 pass: (1,1) first opens each group
                taps(0, [(1, 1), (0, 0), (0, 1), (0, 2), (1, 0), (1, 2),
                         (2, 0), (2, 1), (2, 2)])
                fillers[0]()
                # ci1 pass: (1,1) last closes each group
                taps(1, [(0, 0), (0, 1), (0, 2), (1, 0), (1, 2),
                         (2, 0), (2, 1), (2, 2), (1, 1)])
                for g in range(NG):
                    ot = opool.tile([128, NF], dt.float32, tag="ot")
                    nc.scalar.activation(out=ot[:], in_=pss[g][:], func=AF.Copy)
                    eng = nc.sync if g % 2 == 0 else nc.scalar
                    r0 = g * ROWS
                    eng.dma_start(
                        out=y_d.ap()[s, oh * 128 : (oh + 1) * 128, r0 : r0 + ROWS, :],
                        in_=ot[:].rearrange("p (r w) -> p r w", r=ROWS, w=W),
                    )
                fillers[1]()

            def nothing():
                pass

            # ---- software-pipelined emission ----
            load_consts()
            stage(0)
            load_w()
            stage(1)
            gap_route(0)
            synth(0, 0, 0)
            synth(0, 0, 1)
            for s in range(BL):
                nxt = s + 1
                have_next = nxt < BL
                conv_oh(s, 0, [
                    lambda s=s: synth(s, 1, 0),
                    lambda s=s: synth(s, 1, 1),
                ])
                conv_oh(s, 1, [
                    (lambda n=nxt: (gap_route(n), synth(n, 0, 0)))
                    if have_next else nothing,
                    (lambda n=nxt: (synth(n, 0, 1), stage(n + 1)))
                    if have_next else nothing,
                ])

    nc.compile()
    return nc


def _get_nc():
    if "nc" not in _CACHE:
        _CACHE["nc"] = _build()
    return _CACHE["nc"]


def _pack_inputs(x, kernel_weights, fc_w, fc_b):
    # w layout per partition p (= i % 128): [e, oh, ci, kh, kw, oin], bf16
    a = np.asarray(kernel_weights, np.float32).reshape(E, 2, 128, 2, 128, 3, 3)
    # dims: e, oh, oin, ci, p, kh, kw -> p, e, oh, ci, kh, kw, oin
    a = np.ascontiguousarray(a.transpose(4, 0, 1, 3, 5, 6, 2)).reshape(128, E * EBLK)
    wp = a.astype(ml_dtypes.bfloat16)
    fcw_t = np.ascontiguousarray(np.asarray(fc_w, np.float32).T / float(H * W))
    fcb2 = np.ascontiguousarray(np.asarray(fc_b, np.float32).reshape(E, 1))
    eye = np.eye(E, dtype=np.float32)
    x = np.ascontiguousarray(np.asarray(x, np.float32))
    in_maps = []
    for i in range(N_CORES):
        in_maps.append(
            {
                "x": x[i * BL : (i + 1) * BL],
                "wp": wp,
                "fcw": fcw_t,
                "fcb": fcb2,
                "eye": eye,
            }
        )
    return in_maps


def _run(x, kernel_weights, fc_w, fc_b, trace=False):
    from concourse.bass_utils import run_bass_kernel_spmd

    nc = _get_nc()
    in_maps = _pack_inputs(x, kernel_weights, fc_w, fc_b)
    res = run_bass_kernel_spmd(nc, in_maps, core_ids=list(range(N_CORES)), trace=trace)
    y = np.concatenate([res.results[i]["y"] for i in range(N_CORES)], axis=0)
    return np.ascontiguousarray(y.astype(np.float32)), res


def kernel(x, kernel_weights, fc_w, fc_b):
    y, _ = _run(x, kernel_weights, fc_w, fc_b, trace=False)
    return y


def kernel_traced(x, kernel_weights, fc_w, fc_b):
    y, res = _run(x, kernel_weights, fc_w, fc_b, trace=True)
    return y, res


# revision 10
# speedup vs baseline: 1.4918x; 1.2124x over previous
"""CondConv (per-sample dynamic conv) Trainium2 Bass kernel.

Reference computation (per sample b):
    gap     = mean(x[b], spatial)                    # [C]
    r       = sigmoid(fc_w @ gap + fc_b)             # [E]
    comb    = sum_e r[e] * kernel_weights[e]         # [O, I, 3, 3]
    y[b]    = conv2d(x[b], comb, pad=1)              # [O, H, W]

Sharding: data-parallel over batch, 4 samples per core on 8 cores.
Expert kernels + fc params replicated to every core.

Direct bf16 conv (measured: PE issues one 448-free matmul per ~199ns,
so the 2*7*18 = 252 matmuls/sample are the machine floor; Winograd
trades below-floor PE work for DVE work it cannot afford). The version
differences vs the naive pipeline are all about keeping PE fed from
t=~20us on:

  - x arrives via gpsimd SWDGE cast-DMA (fp32 HBM -> bf16 SBUF,
    contiguous, no padding). Conv edge taps are handled by accumulating
    shifted partial PSUM ranges instead of padding, so there is no
    fp32 staging tile, no ACT cast pass, and no memsets.
  - W (9.4MB) is pushed as 32 (oh, ci, e) chunks alternating across
    BOTH HWDGE rings (sync+scalar), (oh0,ci0) chunks first: the first
    synthesis needs only 2.4MB landed rather than the full 9.4MB.
  - each conv oh-pass opens all 7 PSUM row-groups and runs a ci0 tap
    pass (63 matmuls) then a ci1 pass; sample 0's first synthesis is
    ci-split so conv starts after half a synthesis (~8us) instead of a
    full one.
  - GAP rides a DVE tensor_scalar copy via accum_out (~1us/half);
    routing is PE (fc matmul) -> ACT sigmoid -> PE (eye broadcast).
  - synthesis (DVE tensor_scalar 4x + tensor_tensor 2x over the bf16
    expert stack) is emitted ci-half at a time into 4 filler slots per
    sample, sized so every half lands >=8us before its conv pass.
  - PSUM->SBUF output copies on ACT, output DMA alternates rings.
"""

import numpy as np
import ml_dtypes

B, C, H, W = 32, 256, 56, 56
E = 8
N_CORES = 8
BL = B // N_CORES          # local batch per core
HWU = H * W                # 3136
ROWS = 8                   # output rows per PSUM group
NG = H // ROWS             # 7 groups per oh pass
NF = ROWS * W              # 448 matmul free dim
OIN = 128                  # output channels per half
EBLK = 2 * 2 * 9 * OIN     # per-partition free elems per expert = 4608
OHBLK = EBLK // 2          # per (oh) block = 2304
CIBLK = OHBLK // 2         # per (oh, ci) block = 1152
WP = W + 2                 # col-padded width = 58

_CACHE = {}


def _build():
    import concourse.bacc as bacc
    import concourse.mybir as mybir
    import concourse.tile as tile
    from contextlib import ExitStack

    dt = mybir.dt
    AF = mybir.ActivationFunctionType
    Alu = mybir.AluOpType

    nc = bacc.Bacc(
        "TRN2",
        target_bir_lowering=False,
        debug=False,
        enable_asserts=False,
        num_devices=N_CORES,
    )
    x_d = nc.dram_tensor("x", [BL, C, H, W], dt.float32, kind="ExternalInput")
    w_d = nc.dram_tensor("wp", [128, E * EBLK], dt.bfloat16, kind="ExternalInput")
    fcw_d = nc.dram_tensor("fcw", [C, E], dt.float32, kind="ExternalInput")
    fcb_d = nc.dram_tensor("fcb", [E, 1], dt.float32, kind="ExternalInput")
    eye_d = nc.dram_tensor("eye", [E, E], dt.float32, kind="ExternalInput")
    y_d = nc.dram_tensor("y", [BL, C, H, W], dt.float32, kind="ExternalOutput")

    with tile.TileContext(nc) as tc:
        with ExitStack() as ctx:
            cpool = ctx.enter_context(tc.tile_pool(name="consts", bufs=1))
            xvpool = ctx.enter_context(tc.tile_pool(name="xvs", bufs=3))
            xppool = ctx.enter_context(tc.tile_pool(name="xps", bufs=2))
            cbpool = ctx.enter_context(tc.tile_pool(name="cbs", bufs=2))
            opool = ctx.enter_context(tc.tile_pool(name="outs", bufs=3))
            spool = ctx.enter_context(tc.tile_pool(name="small", bufs=2))
            pspool = ctx.enter_context(tc.tile_pool(name="cpsum", bufs=7, space="PSUM"))
            psmall = ctx.enter_context(tc.tile_pool(name="spsum", bufs=1, space="PSUM"))

            w_sb = cpool.tile([128, E * EBLK], dt.bfloat16)
            fcw_sb = cpool.tile([128, 2 * E], dt.float32)
            fcb_sb = cpool.tile([E, 1], dt.float32)
            eye_sb = cpool.tile([E, E], dt.float32)

            xvs, xps, gaps, rbs, cbs = {}, {}, {}, {}, {}

            def load_consts():
                for ci in range(2):
                    nc.sync.dma_start(
                        out=fcw_sb[:, ci * E : (ci + 1) * E],
                        in_=fcw_d.ap()[ci * 128 : (ci + 1) * 128, :],
                    )
                nc.scalar.dma_start(out=fcb_sb[:], in_=fcb_d.ap())
                nc.scalar.dma_start(out=eye_sb[:], in_=eye_d.ap())

            def load_w():
                # (oh, ci, e) chunks, oh0-ci0 first, round-robin over all
                # three DMA queues (sync, scalar, gpsimd-behind-x0) so the
                # 9.4MB isn't starved by the x cast-DMAs.
                rings = [nc.sync, nc.scalar, nc.gpsimd]
                k = 0
                for oh in range(2):
                    for ci in range(2):
                        for e in range(E):
                            lo = e * EBLK + oh * OHBLK + ci * CIBLK
                            rings[k % 3].dma_start(
                                out=w_sb[:, lo : lo + CIBLK],
                                in_=w_d.ap()[:, lo : lo + CIBLK],
                            )
                            k += 1

            def stage(s):
                if s >= BL:
                    return
                xv = xvpool.tile([128, 2 * HWU], dt.bfloat16, tag="xv", name=f"xv{s}")
                xvs[s] = xv
                for ci in range(2):
                    nc.gpsimd.dma_start(
                        out=xv[:, ci * HWU : (ci + 1) * HWU],
                        in_=x_d.ap()[s, ci * 128 : (ci + 1) * 128, :, :],
                    )

            def gap_route(s):
                # col-padded bf16 copy of x (58-wide rows, zero cols 0/57) so
                # every kw tap is a full-width contiguous-dst matmul; the GAP
                # sum rides the same DVE tensor_scalar via accum_out.
                xv = xvs[s]
                xp = xppool.tile(
                    [128, 2 * H * WP], dt.bfloat16, tag="xp", name=f"xp{s}"
                )
                xps[s] = xp
                xpv = xp.rearrange("p (c h w) -> p c h w", c=2, h=H, w=WP)
                g = spool.tile([128, 2], dt.float32, tag="gap")
                gaps[s] = g
                for ci in range(2):
                    nc.vector.memset(xpv[:, ci, :, 0:1], 0.0)
                    nc.vector.memset(xpv[:, ci, :, WP - 1 : WP], 0.0)
                    nc.vector.tensor_scalar(
                        out=xpv[:, ci, :, 1 : 1 + W],
                        in0=xv[:, ci * HWU : (ci + 1) * HWU],
                        scalar1=1.0,
                        scalar2=0.0,
                        op0=Alu.mult,
                        op1=Alu.add,
                        accum_out=g[:, ci : ci + 1],
                    )
                prt = psmall.tile([128, E], dt.float32, tag="prt", name=f"prt{s}")
                for ci in range(2):
                    nc.tensor.matmul(
                        prt[0:E, 0:1],
                        lhsT=fcw_sb[:, ci * E : (ci + 1) * E],
                        rhs=g[:, ci : ci + 1],
                        start=(ci == 0),
                        stop=(ci == 1),
                    )
                rr = spool.tile([E, 1], dt.float32, tag="rr")
                nc.scalar.activation(
                    out=rr[:], in_=prt[0:E, 0:1], func=AF.Sigmoid, bias=fcb_sb[:],
                    scale=1.0,
                )
                nc.tensor.matmul(
                    prt[:],
                    lhsT=rr[:].broadcast_to([E, 128]),
                    rhs=eye_sb[:],
                    start=True,
                    stop=True,
                )
                rb = spool.tile([128, E], dt.float32, tag="rb")
                nc.scalar.activation(out=rb[:], in_=prt[:], func=AF.Copy)
                rbs[s] = rb

            def synth(s, oh, ci):
                # combined[oh,ci] = sum_e r_e * W_e[oh,ci]  (bf16, DVE)
                if s not in cbs:
                    cbs[s] = cbpool.tile([128, EBLK], dt.bfloat16, tag="cb",
                                         name=f"cb{s}")
                cb = cbs[s]
                rb = rbs[s]
                lo = oh * OHBLK + ci * CIBLK
                dstc = cb[:, lo : lo + CIBLK]
                for e in range(E):
                    src = w_sb[:, e * EBLK + lo : e * EBLK + lo + CIBLK]
                    if e == 0:
                        nc.vector.tensor_scalar_mul(dstc, src, rb[:, 0:1])
                    else:
                        tmp = spool.tile([128, CIBLK], dt.bfloat16, tag="stmp")
                        nc.vector.tensor_scalar_mul(tmp[:], src, rb[:, e : e + 1])
                        nc.vector.tensor_tensor(
                            out=dstc, in0=tmp[:], in1=dstc, op=Alu.add
                        )

            def conv_oh(s, oh, fillers):
                # 7 PSUM groups opened by a ci0 tap pass, closed by ci1.
                # Edge taps (kh at first/last group, kw 0/2 everywhere) write
                # shifted partial ranges; (kh=1,kw=1) is full-range and
                # carries the start/stop flags.
                cb = cbs[s]
                xpv = xps[s].rearrange("p (c h w) -> p c h w", c=2, h=H, w=WP)
                pss = [
                    pspool.tile([128, NF], dt.float32, tag="ps", name=f"ps{s}_{oh}_{g}")
                    for g in range(NG)
                ]

                def taps(ci, order):
                    for g in range(NG):
                        ps = pss[g]
                        psv = ps.rearrange("p (r w) -> p r w", r=ROWS, w=W)
                        r0 = g * ROWS
                        for kh, kw in order:
                            lo = oh * OHBLK + ci * CIBLK + (kh * 3 + kw) * OIN
                            rl = 1 if (g == 0 and kh == 0) else 0
                            rh = ROWS - 1 if (g == NG - 1 and kh == 2) else ROWS
                            full = rl == 0 and rh == ROWS
                            nc.tensor.matmul(
                                ps[:] if full else psv[:, rl:rh, :],
                                lhsT=cb[:, lo : lo + OIN],
                                rhs=xpv[
                                    :,
                                    ci,
                                    r0 + rl + kh - 1 : r0 + rh + kh - 1,
                                    kw : kw + W,
                                ],
                                start=(full and ci == 0 and kh == 1 and kw == 1),
                                stop=(full and ci == 1 and kh == 1 and kw == 1),
                                skip_group_check=not full,
                            )

                # ci0 pass: (1,1) first opens each group
                taps(0, [(1, 1), (0, 0), (0, 1), (0, 2), (1, 0), (1, 2),
                         (2, 0), (2, 1), (2, 2)])
                fillers[0]()
                # ci1 pass: (1,1) last closes each group
                taps(1, [(0, 0), (0, 1), (0, 2), (1, 0), (1, 2),
                         (2, 0), (2, 1), (2, 2), (1, 1)])
                for g in range(NG):
                    ot = opool.tile([128, NF], dt.float32, tag="ot")
                    nc.scalar.activation(out=ot[:], in_=pss[g][:], func=AF.Copy)
                    eng = nc.sync if g % 2 == 0 else nc.scalar
                    r0 = g * ROWS
                    eng.dma_start(
                        out=y_d.ap()[s, oh * 128 : (oh + 1) * 128, r0 : r0 + ROWS, :],
                        in_=ot[:].rearrange("p (r w) -> p r w", r=ROWS, w=W),
                    )
                fillers[1]()

            def nothing():
                pass

            # ---- software-pipelined emission ----
            load_consts()
            stage(0)
            load_w()
            stage(1)
            gap_route(0)
            synth(0, 0, 0)
            synth(0, 0, 1)
            for s in range(BL):
                nxt = s + 1
                have_next = nxt < BL
                conv_oh(s, 0, [
                    lambda s=s: synth(s, 1, 0),
                    lambda s=s: synth(s, 1, 1),
                ])
                conv_oh(s, 1, [
                    (lambda n=nxt: (gap_route(n), synth(n, 0, 0)))
                    if have_next else nothing,
                    (lambda n=nxt: (synth(n, 0, 1), stage(n + 1)))
                    if have_next else nothing,
                ])

    nc.compile()
    return nc


def _get_nc():
    if "nc" not in _CACHE:
        _CACHE["nc"] = _build()
    return _CACHE["nc"]


def _pack_inputs(x, kernel_weights, fc_w, fc_b):
    # w layout per partition p (= i % 128): [e, oh, ci, kh, kw, oin], bf16
    a = np.asarray(kernel_weights, np.float32).reshape(E, 2, 128, 2, 128, 3, 3)
    # dims: e, oh, oin, ci, p, kh, kw -> p, e, oh, ci, kh, kw, oin
    a = np.ascontiguousarray(a.transpose(4, 0, 1, 3, 5, 6, 2)).reshape(128, E * EBLK)
    wp = a.astype(ml_dtypes.bfloat16)
    fcw_t = np.ascontiguousarray(np.asarray(fc_w, np.float32).T / float(H * W))
    fcb2 = np.ascontiguousarray(np.asarray(fc_b, np.float32).reshape(E, 1))
    eye = np.eye(E, dtype=np.float32)
    x = np.ascontiguousarray(np.asarray(x, np.float32))
    in_maps = []
    for i in range(N_CORES):
        in_maps.append(
            {
                "x": x[i * BL : (i + 1) * BL],
                "wp": wp,
                "fcw": fcw_t,
                "fcb": fcb2,
                "eye": eye,
            }
        )
    return in_maps


def _run(x, kernel_weights, fc_w, fc_b, trace=False):
    from concourse.bass_utils import run_bass_kernel_spmd

    nc = _get_nc()
    in_maps = _pack_inputs(x, kernel_weights, fc_w, fc_b)
    res = run_bass_kernel_spmd(nc, in_maps, core_ids=list(range(N_CORES)), trace=trace)
    y = np.concatenate([res.results[i]["y"] for i in range(N_CORES)], axis=0)
    return np.ascontiguousarray(y.astype(np.float32)), res


def kernel(x, kernel_weights, fc_w, fc_b):
    y, _ = _run(x, kernel_weights, fc_w, fc_b, trace=False)
    return y


def kernel_traced(x, kernel_weights, fc_w, fc_b):
    y, res = _run(x, kernel_weights, fc_w, fc_b, trace=True)
    return y, res
